# revision 12
# baseline (speedup 1.0000x reference)
"""Trainium2 Bass kernel for nn_Detector (region-sum pooling + softmax), v7.

out[b,k] = softmax_k( sum of x[b] over region k ), regions = 10 disjoint
113x113 rectangles of the 1024x1024 image.  Data-parallel over batch:
8 NeuronCores x 16 images.

Design (from HW DMA microbenchmarks under full 8-core load):
- The dense [B,HW]x[HW,10] GEMM is a sparse pooling; only 8.10 MB/core
  (the exact 452B-wide region rows) is ever read.  Exact-width 452B
  descriptors run at the same ~200-235 GB/s as padded-512B ones (no
  sub-512B RMW cliff on HBM reads), so no column padding.
- The stream is packet-overhead-bound (~150ns per ~4KB SWDGE packet
  caps it at ~205-235 GB/s; only >4KB descriptors go faster, and any
  wider/contiguous load inflates bytes more than it gains rate).
- SDMA only starts draining a DMA after its full descriptor emission
  (doorbell); Sync's RTL (HWDGE) descriptor generation beats GpSimd's
  first SWDGE doorbell, so region 0's first rows ride the sync queue
  to open the stream earlier.
- The last bulk DMA is a 2-row sliver too: the tail after the final
  byte is just a tiny reduce + matmul + softmax.  Region 9 streams as
  6+6+2 rows with incremental PSUM accumulation (vector + scalar-ACT).
- Remainder row (113th) and blk DMAs issue right after so their
  1-descriptor HWDGE packets drain before the bulk stream ramps.
- Compute (VectorE reduces -> TensorE matmul with a 0/1 octet->batch
  indicator into PSUM; ScalarE softmax) measurably does not slow the
  stream.  One PSUM accumulation group: an identity matmul folds the
  remainder partials in and opens the group early; region matmuls
  accumulate; the final sliver's matmul closes it, so the tail is just
  sliver-reduce -> matmul -> softmax -> store.
"""

import numpy as np

import concourse.bass as bass
import concourse.tile as tile
from concourse import bacc, mybir
from concourse.bass_utils import run_bass_kernel_spmd

B, H, W = 128, 1024, 1024
S = 113  # min(1024 // 9, 1024 // 7)
REGIONS = [(2, 1), (2, 4), (2, 7), (4, 1), (4, 3), (4, 5), (4, 7), (6, 1), (6, 4), (6, 7)]
K = len(REGIONS)
N_CORES = 8
BPC = B // N_CORES
F32 = mybir.dt.float32
OCT, GR = 8, 14  # 112 of the 113 region rows = 8 octets x 14 rows

# Remainder-row groups (row r = rb*S + 112), one stepped-slice HWDGE DMA
# per column block: (row_slice, col_block, [region ks], engine).
REM_GROUPS = [
    ((338, 791, 226), 1, [0, 3, 7], "sync"),
    ((338, 791, 226), 7, [2, 6, 9], "scalar"),
    ((338, 791, 452), 4, [1, 8], "sync"),
    ((564, 565, 1), 3, [4], "scalar"),
    ((564, 565, 1), 5, [5], "sync"),
]


def build_nc():
    nc = bacc.Bacc("TRN2", target_bir_lowering=False, debug=False)
    x = nc.declare_dram_parameter("x", [BPC, H, W], F32, isOutput=False)
    blk_d = nc.declare_dram_parameter("blk", [128, BPC], F32, isOutput=False)
    eye_d = nc.declare_dram_parameter("eye", [BPC, BPC], F32, isOutput=False)
    out = nc.declare_dram_parameter("out", [BPC, K], F32, isOutput=True)

    with tile.TileContext(nc) as tc:
        with (
            tc.tile_pool(name="reg", bufs=1) as rpool,
            tc.tile_pool(name="mp", bufs=1) as mpool,
            tc.tile_pool(name="small", bufs=1) as spool,
            tc.tile_pool(name="psum", bufs=1, space=bass.MemorySpace.PSUM) as ppool,
        ):
            # Bulk stream, exact 452B descriptors.  Region 0's first 4 rows
            # ride the sync HWDGE queue: Sync's RTL descriptor generation
            # beats GpSimd's first SWDGE doorbell by ~1.5us, so these bytes
            # flow while Q0 is still spinning up.  Row-piece plan: region 0
            # = [4 on HWDGE, 10 on Q0], region 9 = [6,6,2] (small final
            # sliver, reduces split across engines), others whole.
            mts = []
            for k, (rb, cb) in enumerate(REGIONS):
                mt = rpool.tile([128, GR, S], F32, name=f"mt{k}", tag=f"mt{k}")
                mts.append(mt)
            r0, c0 = REGIONS[0][0] * S, REGIONS[0][1] * S
            nc.sync.dma_start(out=mts[0][:, 0:4], in_=x[:, r0:r0 + 32, c0:c0 + S])
            pieces = {0: [(4, 10)], K - 1: [(0, 6), (6, 6), (12, 2)]}
            for k, (rb, cb) in enumerate(REGIONS):
                r0, c0 = rb * S, cb * S
                for g0, rows in pieces.get(k, [(0, GR)]):
                    nc.gpsimd.dma_start(
                        out=mts[k][:, g0:g0 + rows],
                        in_=x[:, r0 + g0 * OCT:r0 + (g0 + rows) * OCT, c0:c0 + S],
                        single_packet=True)

            blk = spool.tile([128, BPC], F32)
            nc.sync.dma_start(out=blk[:], in_=blk_d[:])
            eye = spool.tile([BPC, BPC], F32)
            nc.scalar.dma_start(out=eye[:], in_=eye_d[:])

            # Remainder rows (tiny HWDGE transfers).
            rem_tiles = []  # (tile, j, k)
            for (ra, rb_, rs), cb, ks, qname in REM_GROUPS:
                c0 = cb * S
                rt = spool.tile([BPC, len(ks), S], F32, name=f"rem{cb}", tag=f"rem{cb}")
                getattr(nc, qname).dma_start(out=rt[:], in_=x[:, ra:rb_:rs, c0:c0 + S])
                for j, k in enumerate(ks):
                    rem_tiles.append((rt, j, k))

            # Remainder-row partials early (overlap the stream).
            rpart = spool.tile([BPC, K], F32)
            for rt, j, k in sorted(rem_tiles, key=lambda t: t[2]):
                nc.vector.reduce_sum(
                    out=rpart[:, k:k + 1], in_=rt[:, j, :],
                    axis=mybir.AxisListType.X)

            # Per-region reduce -> matmul into PSUM column k.  One PSUM
            # accumulation group: the identity matmul folds the remainder
            # partials in and OPENS the group (start=True, early in the
            # stream); region matmuls accumulate; the final sliver closes.
            py = ppool.tile([BPC, K], F32)
            nc.tensor.matmul(py[:, 0:K], eye[:], rpart[:], start=True, stop=False)
            for k in range(K):
                if k == K - 1:
                    # 6 rows on vector, 6 on scalar-ACT, final 2-row sliver
                    # on vector; three matmuls accumulate in PSUM.
                    mpv = mpool.tile([128, 1], F32, tag="mpv")
                    nc.vector.reduce_sum(
                        out=mpv[:], in_=mts[k][:, 0:6], axis=mybir.AxisListType.XY)
                    scr = mpool.tile([128, 6, S], F32, tag="scr")
                    mpa = mpool.tile([128, 1], F32, tag="mpa")
                    nc.scalar.activation(
                        scr[:], mts[k][:, 6:12],
                        mybir.ActivationFunctionType.Copy, accum_out=mpa[:])
                    mpc = mpool.tile([128, 1], F32, tag="mpc")
                    nc.vector.reduce_sum(
                        out=mpc[:], in_=mts[k][:, 12:GR], axis=mybir.AxisListType.XY)
                    nc.tensor.matmul(py[:, k:k + 1], blk[:], mpv[:], start=False, stop=False)
                    nc.tensor.matmul(py[:, k:k + 1], blk[:], mpa[:], start=False, stop=False)
                    nc.tensor.matmul(py[:, k:k + 1], blk[:], mpc[:], start=False, stop=True)
                else:
                    mp = mpool.tile([128, 1], F32, name=f"mp{k}", tag=f"mp{k}")
                    nc.vector.reduce_sum(
                        out=mp[:], in_=mts[k][:], axis=mybir.AxisListType.XY)
                    nc.tensor.matmul(py[:, k:k + 1], blk[:], mp[:], start=False, stop=False)

            # Stable softmax straight out of PSUM; negated max fused into
            # the reduce.
            negm = spool.tile([BPC, 1], F32)
            nc.vector.tensor_reduce(
                out=negm[:], in_=py[:], axis=mybir.AxisListType.X,
                op=mybir.AluOpType.max, negate=True)
            e = spool.tile([BPC, K], F32)
            ssum = spool.tile([BPC, 1], F32)
            nc.scalar.activation(
                e[:], py[:], mybir.ActivationFunctionType.Exp,
                bias=negm[:], accum_out=ssum[:])
            rcp = spool.tile([BPC, 1], F32)
            nc.vector.reciprocal(rcp[:], ssum[:])
            o = spool.tile([BPC, K], F32)
            nc.vector.tensor_scalar_mul(o[:], e[:], rcp[:])
            nc.sync.dma_start(out=out[:], in_=o[:], single_packet=True)

    nc.compile()
    return nc


_NC = None


def get_nc():
    global _NC
    if _NC is None:
        _NC = build_nc()
    return _NC


def make_in_maps(x):
    blk = np.repeat(np.eye(BPC, dtype=np.float32), OCT, axis=0)
    eye = np.eye(BPC, dtype=np.float32)
    return [
        {"x": np.ascontiguousarray(x[i * BPC:(i + 1) * BPC]), "blk": blk, "eye": eye}
        for i in range(N_CORES)
    ]


def kernel(x, filt=None, **_unused):
    nc = get_nc()
    x = np.ascontiguousarray(np.asarray(x, dtype=np.float32))
    assert x.shape == (B, H, W), x.shape
    try:
        res = run_bass_kernel_spmd(nc, make_in_maps(x), list(range(N_CORES)))
    except Exception:
        # transient device errors (e.g. NRT_EXEC_UNIT_UNRECOVERABLE) clear
        # on re-execution
        res = run_bass_kernel_spmd(nc, make_in_maps(x), list(range(N_CORES)))
    return np.concatenate([r["out"] for r in res.results], axis=0)



# revision 13
# speedup vs baseline: 1.0390x; 1.0390x over previous
"""Trainium2 Bass kernel for nn_Detector (region-sum pooling + softmax), v7.

out[b,k] = softmax_k( sum of x[b] over region k ), regions = 10 disjoint
113x113 rectangles of the 1024x1024 image.  Data-parallel over batch:
8 NeuronCores x 16 images.

Design (from HW DMA microbenchmarks under full 8-core load):
- The dense [B,HW]x[HW,10] GEMM is a sparse pooling; only 8.10 MB/core
  (the exact 452B-wide region rows) is ever read.  Exact-width 452B
  descriptors run at the same ~200-235 GB/s as padded-512B ones (no
  sub-512B RMW cliff on HBM reads), so no column padding.
- The stream is packet-overhead-bound (~150ns per ~4KB SWDGE packet
  caps it at ~205-235 GB/s; only >4KB descriptors go faster, and any
  wider/contiguous load inflates bytes more than it gains rate).
- SDMA only starts draining a DMA after its full descriptor emission
  (doorbell); Sync's RTL (HWDGE) descriptor generation beats GpSimd's
  first SWDGE doorbell, so region 0's first rows ride the sync queue
  to open the stream earlier.
- The last bulk DMA is a 2-row sliver too: the tail after the final
  byte is just a tiny reduce + matmul + softmax.  Region 9 streams as
  6+6+2 rows with incremental PSUM accumulation (vector + scalar-ACT).
- Remainder row (113th) and blk DMAs issue right after so their
  1-descriptor HWDGE packets drain before the bulk stream ramps.
- Compute (VectorE reduces -> TensorE matmul with a 0/1 octet->batch
  indicator into PSUM; ScalarE softmax) measurably does not slow the
  stream.  One PSUM accumulation group: an identity matmul folds the
  remainder partials in and opens the group early; region matmuls
  accumulate; the final sliver's matmul closes it, so the tail is just
  sliver-reduce -> matmul -> softmax -> store.
"""

import numpy as np

import concourse.bass as bass
import concourse.tile as tile
from concourse import bacc, mybir
from concourse.bass_utils import run_bass_kernel_spmd

B, H, W = 128, 1024, 1024
S = 113  # min(1024 // 9, 1024 // 7)
REGIONS = [(2, 1), (2, 4), (2, 7), (4, 1), (4, 3), (4, 5), (4, 7), (6, 1), (6, 4), (6, 7)]
K = len(REGIONS)
N_CORES = 8
BPC = B // N_CORES
F32 = mybir.dt.float32
OCT, GR = 8, 14  # 112 of the 113 region rows = 8 octets x 14 rows

# Remainder-row groups (row r = rb*S + 112), one stepped-slice HWDGE DMA
# per column block: (row_slice, col_block, [region ks], engine).
REM_GROUPS = [
    ((338, 791, 226), 1, [0, 3, 7], "sync"),
    ((338, 791, 226), 7, [2, 6, 9], "scalar"),
    ((338, 791, 452), 4, [1, 8], "sync"),
    ((564, 565, 1), 3, [4], "scalar"),
    ((564, 565, 1), 5, [5], "sync"),
]


def build_nc():
    nc = bacc.Bacc("TRN2", target_bir_lowering=False, debug=False)
    x = nc.declare_dram_parameter("x", [BPC, H, W], F32, isOutput=False)
    blk_d = nc.declare_dram_parameter("blk", [128, BPC], F32, isOutput=False)
    eye_d = nc.declare_dram_parameter("eye", [BPC, BPC], F32, isOutput=False)
    out = nc.declare_dram_parameter("out", [BPC, K], F32, isOutput=True)

    with tile.TileContext(nc) as tc:
        with (
            tc.tile_pool(name="reg", bufs=1) as rpool,
            tc.tile_pool(name="mp", bufs=1) as mpool,
            tc.tile_pool(name="small", bufs=1) as spool,
            tc.tile_pool(name="psum", bufs=1, space=bass.MemorySpace.PSUM) as ppool,
        ):
            # Bulk SWDGE stream, exact 452B descriptors.  Row-piece plan:
            # region 0 = [2,12] (2-row sliver rings the first doorbell
            # ~1us sooner), region 9 = [6,6,2] (small final sliver,
            # reduces split across engines), others whole.  The tiny blk
            # and eye loads go on the SWDGE queue BEHIND the first bulk
            # DMAs: as 1-descriptor HWDGE packets ahead of Q0's doorbell
            # they would delay the stream start by up to ~3us (measured).
            mts = []
            for k, (rb, cb) in enumerate(REGIONS):
                mt = rpool.tile([128, GR, S], F32, name=f"mt{k}", tag=f"mt{k}")
                mts.append(mt)
            pieces = {0: [(0, 2), (2, 12)], K - 1: [(0, 6), (6, 6), (12, 2)]}
            blk = spool.tile([128, BPC], F32)
            eye = spool.tile([BPC, BPC], F32)
            for k, (rb, cb) in enumerate(REGIONS):
                r0, c0 = rb * S, cb * S
                for g0, rows in pieces.get(k, [(0, GR)]):
                    nc.gpsimd.dma_start(
                        out=mts[k][:, g0:g0 + rows],
                        in_=x[:, r0 + g0 * OCT:r0 + (g0 + rows) * OCT, c0:c0 + S],
                        single_packet=True)
                if k == 0:
                    nc.gpsimd.dma_start(out=blk[:], in_=blk_d[:])
                    nc.gpsimd.dma_start(out=eye[:], in_=eye_d[:])

            # Remainder rows (tiny HWDGE transfers).
            rem_tiles = []  # (tile, j, k)
            for (ra, rb_, rs), cb, ks, qname in REM_GROUPS:
                c0 = cb * S
                rt = spool.tile([BPC, len(ks), S], F32, name=f"rem{cb}", tag=f"rem{cb}")
                getattr(nc, qname).dma_start(out=rt[:], in_=x[:, ra:rb_:rs, c0:c0 + S])
                for j, k in enumerate(ks):
                    rem_tiles.append((rt, j, k))

            # Remainder-row partials early (overlap the stream).
            rpart = spool.tile([BPC, K], F32)
            for rt, j, k in sorted(rem_tiles, key=lambda t: t[2]):
                nc.vector.reduce_sum(
                    out=rpart[:, k:k + 1], in_=rt[:, j, :],
                    axis=mybir.AxisListType.X)

            # Per-region reduce -> matmul into PSUM column k.  One PSUM
            # accumulation group: the identity matmul folds the remainder
            # partials in and OPENS the group (start=True, early in the
            # stream); region matmuls accumulate; the final sliver closes.
            py = ppool.tile([BPC, K], F32)
            nc.tensor.matmul(py[:, 0:K], eye[:], rpart[:], start=True, stop=False)
            for k in range(K):
                if k == K - 1:
                    # 6 rows on vector, 6 on scalar-ACT, final 2-row sliver
                    # on vector; three matmuls accumulate in PSUM.
                    mpv = mpool.tile([128, 1], F32, tag="mpv")
                    nc.vector.reduce_sum(
                        out=mpv[:], in_=mts[k][:, 0:6], axis=mybir.AxisListType.XY)
                    scr = mpool.tile([128, 6, S], F32, tag="scr")
                    mpa = mpool.tile([128, 1], F32, tag="mpa")
                    nc.scalar.activation(
                        scr[:], mts[k][:, 6:12],
                        mybir.ActivationFunctionType.Copy, accum_out=mpa[:])
                    mpc = mpool.tile([128, 1], F32, tag="mpc")
                    nc.vector.reduce_sum(
                        out=mpc[:], in_=mts[k][:, 12:GR], axis=mybir.AxisListType.XY)
                    nc.tensor.matmul(py[:, k:k + 1], blk[:], mpv[:], start=False, stop=False)
                    nc.tensor.matmul(py[:, k:k + 1], blk[:], mpa[:], start=False, stop=False)
                    nc.tensor.matmul(py[:, k:k + 1], blk[:], mpc[:], start=False, stop=True)
                else:
                    mp = mpool.tile([128, 1], F32, name=f"mp{k}", tag=f"mp{k}")
                    nc.vector.reduce_sum(
                        out=mp[:], in_=mts[k][:], axis=mybir.AxisListType.XY)
                    nc.tensor.matmul(py[:, k:k + 1], blk[:], mp[:], start=False, stop=False)

            # Stable softmax straight out of PSUM; negated max fused into
            # the reduce.
            negm = spool.tile([BPC, 1], F32)
            nc.vector.tensor_reduce(
                out=negm[:], in_=py[:], axis=mybir.AxisListType.X,
                op=mybir.AluOpType.max, negate=True)
            e = spool.tile([BPC, K], F32)
            ssum = spool.tile([BPC, 1], F32)
            nc.scalar.activation(
                e[:], py[:], mybir.ActivationFunctionType.Exp,
                bias=negm[:], accum_out=ssum[:])
            rcp = spool.tile([BPC, 1], F32)
            nc.vector.reciprocal(rcp[:], ssum[:])
            o = spool.tile([BPC, K], F32)
            nc.vector.tensor_scalar_mul(o[:], e[:], rcp[:])
            nc.sync.dma_start(out=out[:], in_=o[:], single_packet=True)

    nc.compile()
    return nc


_NC = None


def get_nc():
    global _NC
    if _NC is None:
        _NC = build_nc()
    return _NC


def make_in_maps(x):
    blk = np.repeat(np.eye(BPC, dtype=np.float32), OCT, axis=0)
    eye = np.eye(BPC, dtype=np.float32)
    return [
        {"x": np.ascontiguousarray(x[i * BPC:(i + 1) * BPC]), "blk": blk, "eye": eye}
        for i in range(N_CORES)
    ]


def kernel(x, filt=None, **_unused):
    nc = get_nc()
    x = np.ascontiguousarray(np.asarray(x, dtype=np.float32))
    assert x.shape == (B, H, W), x.shape
    try:
        res = run_bass_kernel_spmd(nc, make_in_maps(x), list(range(N_CORES)))
    except Exception:
        # transient device errors (e.g. NRT_EXEC_UNIT_UNRECOVERABLE) clear
        # on re-execution
        res = run_bass_kernel_spmd(nc, make_in_maps(x), list(range(N_CORES)))
    return np.concatenate([r["out"] for r in res.results], axis=0)

